# revision 1
# baseline (speedup 1.0000x reference)
"""CPFGNN Trainium2 kernel: 8-core SPMD Bass implementation.

Math (exact simplifications of the reference):
  - lam = 2.0 always (w_off <= 0), so diag = 0 and prop(t) is a pure
    edge scatter-add: prop(t) = -D^-1/2 A^T D^-1/2 t, with A the
    (multi-)adjacency count matrix excluding self-loops and deg = out-degree.
  - The 11 CTC @ e_k matvecs batch into one CTC @ E (N x 11) pass.

Mapping:
  - Nodes sharded 8 ways (1250 rows each): feature/CTC/A/output row-sharded.
  - prop is computed DENSELY on the TensorEngine: A is shipped as an exact
    fp8(e4m3) count matrix (values 0,1,2.. are exact); the moving operand
    streams A column-blocks; the stationary operand is u = D^-1/2 t split
    into fp8 hi+lo columns (M=20), recovering ~bf16 accuracy.
  - Per hop: 25 KB fp8 AllGather of each core's u-block.
  - CTC is shipped pre-transposed in bf16 and streamed once as the moving
    operand against the stationary E matrix (M=11).
"""
import os
import sys

sys.path.insert(0, "/opt/trn_rl_repo")

import numpy as np
import ml_dtypes
from contextlib import ExitStack

N = 10000
E_EDGES = 320000
F_IN = 500
HID = 64
C = 10
RANK = 3
K = 10
NC = 8
NSH = N // NC              # 1250 nodes per core
JT = (N + 127) // 128      # 79 src tiles (last partial: 16)
JLAST = N - 128 * (JT - 1)  # 16
LT = (NSH + 127) // 128    # 10 local node tiles (last partial: 98)
LLAST = NSH - 128 * (LT - 1)  # 98
STRIPS = [(0, 512), (512, 512), (1024, NSH - 1024)]
# per-core row blocks: each core's 1250 nodes = 9 full 128-tiles + one 98-tile
GJT = NC * LT               # 80 global j-tiles in per-core-tiled order
UW = 128                    # padded fp8 u row (hi 0:10, mid 32:42, lo 64:74)
EW = 32                     # padded bf16 e row (11 used)

NP_FP8 = ml_dtypes.float8_e4m3
NP_BF16 = ml_dtypes.bfloat16

_CACHE = {}


def _build_program():
    import concourse.bass as bass
    import concourse.tile as tile
    from concourse import bacc, mybir
    from concourse.masks import make_identity

    dt = mybir.dt
    FP8 = dt.float8e4
    BF16 = dt.bfloat16
    FP16 = dt.float16
    F32 = dt.float32
    AF = mybir.ActivationFunctionType
    ALU = mybir.AluOpType

    nc = bacc.Bacc("TRN2", target_bir_lowering=False, debug=False, num_devices=NC)

    # ---------------- DRAM I/O ----------------
    a_dram = nc.dram_tensor("a8", [N, NSH], FP8, kind="ExternalInput")
    featT_dram = nc.dram_tensor("featT", [F_IN, NSH], F32, kind="ExternalInput")
    ctct_dram = nc.dram_tensor("ctct", [N, NSH], FP16, kind="ExternalInput")
    w1_dram = nc.dram_tensor("w1", [F_IN, HID], F32, kind="ExternalInput")
    b1_dram = nc.dram_tensor("b1", [HID, 1], F32, kind="ExternalInput")
    w2_dram = nc.dram_tensor("w2", [HID, C], F32, kind="ExternalInput")
    b2_dram = nc.dram_tensor("b2", [C, 1], F32, kind="ExternalInput")
    wp_dram = nc.dram_tensor("wp", [C, (K + 1) * RANK], FP16, kind="ExternalInput")
    bp_dram = nc.dram_tensor("bp", [RANK, K + 1], F32, kind="ExternalInput")
    gam_dram = nc.dram_tensor("gam", [RANK, K + 1], FP16, kind="ExternalInput")
    # rows: 0 = dinv_loc, 1 = -dinv_loc, 2 = -2*dinv_loc   (this core's range)
    dinv_dram = nc.dram_tensor("dinvs", [3, NSH], F32, kind="ExternalInput")
    sel3_dram = nc.dram_tensor("sel3", [3, 30], F32, kind="ExternalInput")
    sel11_dram = nc.dram_tensor("sel11", [K + 1, (K + 1) * C], F32, kind="ExternalInput")
    out_dram = nc.dram_tensor("out", [NSH, C], F32, kind="ExternalOutput")
    DEBUG = bool(os.environ.get("GNN_DEBUG"))
    if DEBUG:
        dump_tx = nc.dram_tensor("dump_tx", [K + 1, C, NSH], F32, kind="ExternalOutput")
        dump_e = nc.dram_tensor("dump_e", [K + 1, NSH], F32, kind="ExternalOutput")
        dump_eta = nc.dram_tensor("dump_eta", [K + 1, NSH], F32, kind="ExternalOutput")
        dump_hid = nc.dram_tensor("dump_hid", [C, NSH], F32, kind="ExternalOutput")
        dump_ustat = nc.dram_tensor("dump_ustat", [128, GJT, UW], F32, kind="ExternalOutput")
        dump_prop = nc.dram_tensor("dump_prop", [C, NSH], F32, kind="ExternalOutput")

    ag_u_in = nc.dram_tensor("ag_u_in", [128, LT, UW], FP8)
    ag_u_out = nc.dram_tensor("ag_u_out", [NC, 128, LT, UW], FP8, addr_space="Shared")
    ag_e_in = nc.dram_tensor("ag_e_in", [128, LT, EW], FP16)
    ag_e_out = nc.dram_tensor("ag_e_out", [NC, 128, LT, EW], FP16, addr_space="Shared")

    RG = [list(range(NC))]


    with ExitStack() as ctx:
        tc = ctx.enter_context(tile.TileContext(nc))
        const = ctx.enter_context(tc.tile_pool(name="const", bufs=1))
        big = ctx.enter_context(tc.tile_pool(name="big", bufs=1))     # [C,NSH]-ish f32 temps
        small = ctx.enter_context(tc.tile_pool(name="small", bufs=3))  # small temps
        stream = ctx.enter_context(tc.tile_pool(name="stream", bufs=3))

        # ------------- resident constants -------------
        w1 = const.tile([128, 4, HID], F32, tag="w1")
        nc.sync.dma_start(
            w1[:, 0:3, :], w1_dram[0:384, :].rearrange("(t p) c -> p t c", p=128)
        )
        nc.sync.dma_start(w1[0:F_IN - 384, 3, :], w1_dram[384:F_IN, :])
        b1 = const.tile([HID, 1], F32, tag="b1")
        nc.sync.dma_start(b1[:], b1_dram[:])
        w2 = const.tile([HID, C], F32, tag="w2")
        nc.sync.dma_start(w2[:], w2_dram[:])
        b2 = const.tile([C, 1], F32, tag="b2")
        nc.sync.dma_start(b2[:], b2_dram[:])
        wp = const.tile([C, (K + 1) * RANK], FP16, tag="wp")
        nc.sync.dma_start(wp[:], wp_dram[:])
        bp = const.tile([RANK, K + 1], F32, tag="bp")
        nc.sync.dma_start(bp[:], bp_dram[:])
        gam = const.tile([RANK, K + 1], FP16, tag="gam")
        nc.sync.dma_start(gam[:], gam_dram[:])
        dinvs = const.tile([3, NSH], F32, tag="dinvs")
        nc.sync.dma_start(dinvs[:], dinv_dram[:])
        sel3 = const.tile([3, 30], F32, tag="sel3")
        nc.sync.dma_start(sel3[:], sel3_dram[:])
        sel11 = const.tile([K + 1, (K + 1) * C], F32, tag="sel11")
        nc.sync.dma_start(sel11[:], sel11_dram[:])
        dB = [const.tile([C, NSH], F32, tag=f"dB{r}", name=f"dB{r}") for r in range(3)]
        with tc.tile_pool(name="psD", bufs=2, space="PSUM") as psD:
            for r in range(3):
                for s0, sw in STRIPS:
                    psd = psD.tile([C, 512], F32, space="PSUM", tag="psd",
                                   name=f"psd{r}")
                    nc.tensor.matmul(psd[:, 0:sw], sel3[:, r * 10:(r + 1) * 10],
                                     dinvs[:, s0:s0 + sw], start=True, stop=True)
                    nc.vector.tensor_copy(dB[r][:, s0:s0 + sw], psd[:, 0:sw])

        ident = const.tile([128, 128], F32, tag="ident")
        make_identity(nc, ident[:])

        u_stat = const.tile([128, GJT, UW], FP8, tag="u_stat")
        e_stat = const.tile([128, GJT, EW], FP16, tag="e_stat")

        # bf16 history of all Tx_k (for eta + hidden), f32 rotation state
        hist = [const.tile([C, NSH], FP16, tag=f"h{k}", name=f"hist{k}")
                for k in range(K + 1)]
        st = [const.tile([C, NSH], F32, tag=f"st{i}", name=f"state{i}")
              for i in range(3)]
        eT = const.tile([K + 1, NSH], F32, tag="eT")
        u_loc8 = const.tile([128, LT, UW], FP8, tag="u_loc8")
        hidT = const.tile([C, NSH], F32, tag="hidT")
        x1T = const.tile([HID, NSH], F32, tag="x1T")

        # ---------------- MLP ----------------
        KT = [(0, 128), (128, 128), (256, 128), (384, F_IN - 384)]
        with tc.tile_pool(name="psmlp", bufs=3, space="PSUM") as psmlp:
            fts = []
            for ki, (k0, kw) in enumerate(KT):
                ft = stream.tile([128, NSH], F32, tag="big", name=f"ft{ki}", bufs=3)
                nc.sync.dma_start(ft[0:kw, :], featT_dram[k0:k0 + kw, :])
                fts.append(ft)
            for s0, sw in STRIPS:
                ps = psmlp.tile([HID, 512], F32, space="PSUM", tag="psA", name="psA")
                for ki, (k0, kw) in enumerate(KT):
                    nc.tensor.matmul(
                        ps[:, 0:sw], w1[0:kw, ki, :], fts[ki][0:kw, s0:s0 + sw],
                        start=(ki == 0), stop=(ki == 3),
                    )
                nc.scalar.activation(x1T[:, s0:s0 + sw], ps[:, 0:sw], AF.Relu,
                                     bias=b1[:], scale=1.0)
            for s0, sw in STRIPS:
                ps2 = psmlp.tile([C, 512], F32, space="PSUM", tag="ps2", name="psB")
                nc.tensor.matmul(ps2[:, 0:sw], w2[:], x1T[:, s0:s0 + sw],
                                 start=True, stop=True)
                nc.scalar.activation(st[0][:, s0:s0 + sw], ps2[:, 0:sw], AF.Identity,
                                     bias=b2[:], scale=1.0)
        nc.vector.tensor_copy(hist[0][:], st[0][:])
        if DEBUG:
            nc.sync.dma_start(dump_tx[0], st[0][:])

        # ---------------- helpers ----------------
        def compute_eta(k):
            """e_k = tanh(Txk @ Wp[k] + bp[k]) @ (gamma[:,k]/3) into eT row k."""
            eRow = small.tile([1, NSH], F32, tag="eRow", name=f"eRow{k}")
            with tc.tile_pool(name=f"pse{k}", bufs=2, space="PSUM") as pse:
                for s0, sw in STRIPS:
                    psh = pse.tile([RANK, 512], F32, space="PSUM", tag="psh",
                                   name=f"psh{k}")
                    nc.tensor.matmul(psh[:, 0:sw], wp[:, k * RANK:(k + 1) * RANK],
                                     hist[k][:, s0:s0 + sw], start=True, stop=True)
                    hta = small.tile([RANK, 512], FP16, tag="hta", name=f"hta{k}")
                    nc.scalar.activation(hta[:, 0:sw], psh[:, 0:sw], AF.Tanh,
                                         bias=bp[:, k:k + 1], scale=1.0)
                    pse2 = pse.tile([1, 512], F32, space="PSUM", tag="pse2",
                                    name=f"pse2{k}")
                    nc.tensor.matmul(pse2[:, 0:sw], gam[:, k:k + 1], hta[:, 0:sw],
                                     start=True, stop=True)
                    nc.vector.tensor_copy(eRow[:, s0:s0 + sw], pse2[:, 0:sw])
            nc.sync.dma_start(eT[k:k + 1, :], eRow[:])

        def prep_u(cur, tag):
            """cur [C, NSH] f32 * dinv -> u_loc8 [128, LT, 20] fp8 hi/lo node-major."""
            uT = big.tile([C, NSH], F32, tag="uT", name=f"uT{tag}")
            nc.vector.tensor_tensor(out=uT[:], in0=cur[:],
                                    in1=dB[0][:], op=ALU.mult)
            with tc.tile_pool(name=f"psu{tag}", bufs=3, space="PSUM") as psu:
                for t in range(LT):
                    pw = 128 if t < LT - 1 else LLAST
                    psT = psu.tile([128, C], F32, space="PSUM", tag="psuT", name=f"psu{tag}_{t}")
                    nc.tensor.transpose(psT[0:pw, :], uT[:, t * 128:t * 128 + pw],
                                        ident[0:C, 0:C])
                    nc.vector.tensor_copy(u_loc8[0:pw, t, 0:10], psT[0:pw, :])
                    hif = small.tile([128, C], F32, tag="hif", name=f"hif{tag}_{t}")
                    nc.scalar.activation(hif[0:pw, :], u_loc8[0:pw, t, 0:10], AF.Copy)
                    r1 = small.tile([128, C], F32, tag="r1", name=f"r1{tag}_{t}")
                    nc.vector.tensor_tensor(out=r1[0:pw, :], in0=psT[0:pw, :],
                                            in1=hif[0:pw, :], op=ALU.subtract)
                    nc.scalar.activation(u_loc8[0:pw, t, 32:42], r1[0:pw, :],
                                         AF.Copy, scale=64.0)
                    midf = small.tile([128, C], F32, tag="midf", name=f"midf{tag}_{t}")
                    nc.scalar.activation(midf[0:pw, :], u_loc8[0:pw, t, 32:42],
                                         AF.Copy, scale=1.0 / 64.0)
                    r2 = small.tile([128, C], F32, tag="r2", name=f"r2{tag}_{t}")
                    nc.vector.tensor_tensor(out=r2[0:pw, :], in0=r1[0:pw, :],
                                            in1=midf[0:pw, :], op=ALU.subtract)
                    nc.scalar.activation(u_loc8[0:pw, t, 64:74], r2[0:pw, :],
                                         AF.Copy, scale=4096.0)

        def allgather_u():
            with tc.tile_critical():
                cc_sem = nc.alloc_semaphore(None)
                dma_sem = nc.alloc_semaphore(None)
                nc.sync.dma_start(out=ag_u_in[:], in_=u_loc8[:]).then_inc(dma_sem, 16)
                nc.sync.wait_ge(dma_sem, 16)
                nc.gpsimd.collective_compute(
                    "AllGather", ALU.bypass, replica_groups=RG,
                    ins=[ag_u_in[:]], outs=[ag_u_out[:]],
                ).then_inc(cc_sem, 1)
                nc.sync.wait_ge(cc_sem, 1)
                nc.sync.dma_start(
                    out=u_stat[:].rearrange("p (c t) x -> p c t x", c=NC),
                    in_=ag_u_out[:].rearrange("c p t x -> p c t x"),
                ).then_inc(dma_sem, 16)
                nc.sync.wait_ge(dma_sem, 32)

        # ---------------- Tx0 prep ----------------
        compute_eta(0)
        prep_u(st[0], "h0")

        # ---------------- hops ----------------
        cur_i, prev_i, free_i = 0, None, 1
        for k in range(1, K + 1):
            allgather_u()
            if DEBUG and k == 2:
                nc.gpsimd.dma_start(dump_ustat[:], u_stat[:])
            with tc.tile_pool(name=f"psh{k}", bufs=1, space="PSUM") as psh:
                pss = []
                for si, (s0, sw) in enumerate(STRIPS):
                    pss.append(psh.tile([74, 512], F32, space="PSUM", tag=f"s{si}",
                                        name=f"hop{k}s{si}"))
                for cg in range(NC):
                    r0 = cg * NSH
                    ach = stream.tile([128, LT, NSH], FP8, tag="big",
                                      name=f"ach{k}_{cg}")
                    nc.sync.dma_start(
                        ach[:, 0:LT - 1, :],
                        a_dram[r0:r0 + 128 * (LT - 1), :]
                        .rearrange("(t p) c -> p t c", p=128),
                    )
                    nc.sync.dma_start(ach[0:LLAST, LT - 1, :],
                                      a_dram[r0 + 128 * (LT - 1):r0 + NSH, :])
                    for t in range(LT):
                        kw = 128 if t < LT - 1 else LLAST
                        jg = cg * LT + t
                        for si, (s0, sw) in enumerate(STRIPS):
                            nc.tensor.matmul(
                                pss[si][:, 0:sw], u_stat[0:kw, jg, 0:74],
                                ach[0:kw, t, s0:s0 + sw],
                                start=(jg == 0), stop=(jg == GJT - 1),
                            )
                propT = big.tile([C, NSH], F32, tag="propT", name=f"propT{k}")
                for si, (s0, sw) in enumerate(STRIPS):
                    hiS = small.tile([C, 512], F32, tag="hiS", name=f"hiS{k}_{si}")
                    nc.vector.tensor_copy(hiS[:, 0:sw], pss[si][0:C, 0:sw])
                    miS = small.tile([C, 512], F32, tag="miS", name=f"miS{k}_{si}")
                    nc.scalar.activation(miS[:, 0:sw], pss[si][32:32 + C, 0:sw],
                                         AF.Copy, scale=1.0 / 64.0)
                    loS = small.tile([C, 512], F32, tag="loS", name=f"loS{k}_{si}")
                    nc.scalar.activation(loS[:, 0:sw], pss[si][64:64 + C, 0:sw],
                                         AF.Copy, scale=1.0 / 4096.0)
                    nc.vector.tensor_tensor(out=hiS[:, 0:sw],
                                            in0=hiS[:, 0:sw],
                                            in1=miS[:, 0:sw], op=ALU.add)
                    nc.vector.tensor_tensor(out=propT[:, s0:s0 + sw],
                                            in0=hiS[:, 0:sw],
                                            in1=loS[:, 0:sw], op=ALU.add)
            if DEBUG and k == 2:
                nc.sync.dma_start(dump_prop[:], propT[:])
            # chebyshev combine into a fresh state tile
            scale_rows = dB[1][:] if k == 1 else dB[2][:]
            scaled = big.tile([C, NSH], F32, tag="scaled", name=f"scaled{k}")
            nc.vector.tensor_tensor(out=scaled[:], in0=propT[:],
                                    in1=scale_rows[:], op=ALU.mult)
            nxt = st[free_i]
            if k == 1:
                nc.vector.tensor_copy(nxt[:], scaled[:])
            else:
                nc.vector.tensor_tensor(out=nxt[:], in0=scaled[:],
                                        in1=st[prev_i][:], op=ALU.subtract)
            nc.vector.tensor_copy(hist[k][:], nxt[:])
            if DEBUG:
                nc.sync.dma_start(dump_tx[k], nxt[:])
            prev_i, cur_i = cur_i, free_i
            free_i = 3 - cur_i - prev_i
            compute_eta(k)
            if k < K:
                prep_u(st[cur_i], f"h{k}")

        if DEBUG:
            nc.sync.dma_start(dump_e[:], eT[:])
        # ---------------- E allgather ----------------
        e_loc = const.tile([128, LT, EW], FP16, tag="e_loc")
        with tc.tile_pool(name="psE", bufs=3, space="PSUM") as psE:
            for t in range(LT):
                pw = 128 if t < LT - 1 else LLAST
                psT = psE.tile([128, K + 1], F32, space="PSUM", tag="psET", name=f"psE{t}")
                nc.tensor.transpose(psT[0:pw, :], eT[:, t * 128:t * 128 + pw],
                                    ident[0:K + 1, 0:K + 1])
                nc.vector.tensor_copy(e_loc[0:pw, t, 0:K + 1], psT[0:pw, :])
        with tc.tile_critical():
            cc_sem = nc.alloc_semaphore(None)
            dma_sem = nc.alloc_semaphore(None)
            nc.sync.dma_start(out=ag_e_in[:], in_=e_loc[:]).then_inc(dma_sem, 16)
            nc.sync.wait_ge(dma_sem, 16)
            nc.gpsimd.collective_compute(
                "AllGather", ALU.bypass, replica_groups=RG,
                ins=[ag_e_in[:]], outs=[ag_e_out[:]],
            ).then_inc(cc_sem, 1)
            nc.sync.wait_ge(cc_sem, 1)
            nc.sync.dma_start(
                out=e_stat[:].rearrange("p (c t) x -> p c t x", c=NC),
                in_=ag_e_out[:].rearrange("c p t x -> p c t x"),
            ).then_inc(dma_sem, 16)
            nc.sync.wait_ge(dma_sem, 32)

        # ---------------- CTC @ E + hidden ----------------
        with tc.tile_pool(name="psC", bufs=1, space="PSUM") as psC:
            pss = [psC.tile([K + 1, 512], F32, space="PSUM", tag=f"c{si}",
                            name=f"ctc{si}") for si in range(3)]
            for jg in range(GJT):
                cg, t = jg // LT, jg % LT
                kw = 128 if t < LT - 1 else LLAST
                row0 = cg * NSH + t * 128
                cj = stream.tile([128, NSH], FP16, tag="big", name=f"cj{jg}")
                nc.sync.dma_start(cj[0:kw, :], ctct_dram[row0:row0 + kw, :])
                for si, (s0, sw) in enumerate(STRIPS):
                    nc.tensor.matmul(
                        pss[si][:, 0:sw], e_stat[0:kw, jg, 0:K + 1],
                        cj[0:kw, s0:s0 + sw],
                        start=(jg == 0), stop=(jg == GJT - 1),
                    )
            # hidden = sum_k TxkT * (row k of Eta replicated to C partitions)
            etaS = big.tile([K + 1, NSH], F32, tag="etaS", name="etaS")
            for si, (s0, sw) in enumerate(STRIPS):
                nc.vector.tensor_copy(etaS[:, s0:s0 + sw], pss[si][:, 0:sw])
            if DEBUG:
                nc.sync.dma_start(dump_eta[:], etaS[:])
            with tc.tile_pool(name="psR", bufs=3, space="PSUM") as psR:
                for si, (s0, sw) in enumerate(STRIPS):
                    for k in range(K + 1):
                        psr = psR.tile([C, 512], F32, space="PSUM", tag="psr",
                                       name=f"psr{si}_{k}")
                        nc.tensor.matmul(psr[:, 0:sw], sel11[:, k * C:(k + 1) * C],
                                         etaS[:, s0:s0 + sw], start=True, stop=True)
                        tmp = small.tile([C, 512], F32, tag="htmp",
                                         name=f"htmp{si}_{k}")
                        nc.vector.tensor_tensor(
                            out=tmp[:, 0:sw], in0=hist[k][:, s0:s0 + sw],
                            in1=psr[:, 0:sw], op=ALU.mult)
                        if k == 0:
                            nc.vector.tensor_copy(hidT[:, s0:s0 + sw], tmp[:, 0:sw])
                        else:
                            nc.vector.tensor_tensor(out=hidT[:, s0:s0 + sw],
                                                    in0=hidT[:, s0:s0 + sw],
                                                    in1=tmp[:, 0:sw], op=ALU.add)

        if DEBUG:
            nc.sync.dma_start(dump_hid[:], hidT[:])
        # ---------------- log_softmax + out ----------------
        with tc.tile_pool(name="psS", bufs=3, space="PSUM") as psS:
            for t in range(LT):
                pw = 128 if t < LT - 1 else LLAST
                psT = psS.tile([128, C], F32, space="PSUM", tag="psST", name=f"psS{t}")
                nc.tensor.transpose(psT[0:pw, :], hidT[:, t * 128:t * 128 + pw],
                                    ident[0:C, 0:C])
                h = small.tile([128, C], F32, tag="hrow", name=f"hrow{t}")
                nc.vector.tensor_copy(h[0:pw, :], psT[0:pw, :])
                mx = small.tile([128, 1], F32, tag="mx", name=f"mx{t}")
                nc.vector.tensor_reduce(mx[0:pw, :], h[0:pw, :],
                                        axis=mybir.AxisListType.X, op=ALU.max)
                sh = small.tile([128, C], F32, tag="sh", name=f"sh{t}")
                nc.vector.tensor_scalar_sub(sh[0:pw, :], h[0:pw, :], mx[0:pw, :])
                ex = small.tile([128, C], F32, tag="ex", name=f"ex{t}")
                sm = small.tile([128, 1], F32, tag="sm", name=f"sm{t}")
                nc.scalar.activation(ex[0:pw, :], sh[0:pw, :], AF.Exp,
                                     accum_out=sm[0:pw, :])
                ls = small.tile([128, 1], F32, tag="ls", name=f"ls{t}")
                nc.scalar.activation(ls[0:pw, :], sm[0:pw, :], AF.Ln)
                o = small.tile([128, C], F32, tag="o", name=f"o{t}")
                nc.vector.tensor_scalar_sub(o[0:pw, :], sh[0:pw, :], ls[0:pw, :])
                nc.sync.dma_start(out_dram[t * 128:t * 128 + pw, :], o[0:pw, :])

    nc.compile()
    return nc


def _host_prep(feature, edges, CTC, W1, b1, W2, b2, gamma, Wp, bp):
    src = np.asarray(edges[0], dtype=np.int64)
    dst = np.asarray(edges[1], dtype=np.int64)
    nonself = src != dst
    s, d = src[nonself], dst[nonself]

    deg = np.bincount(s, minlength=N).astype(np.float64)
    dinv = np.where(deg > 0, 1.0 / np.sqrt(np.maximum(deg, 1e-30)), 0.0).astype(np.float32)

    counts = np.zeros((N, N), dtype=np.uint8)
    np.add.at(counts, (s, d), 1)
    lut = np.arange(256).astype(NP_FP8)
    a8 = lut[counts]          # [N, N] fp8, exact small ints
    # per-core-tiled row order: for core c, tiles of 128 (last 98); this is
    # just the identity permutation within each core range, concatenated - the
    # rows are already in that order, so no permutation needed. (Row blocks
    # are consecutive: core c rows [1250c, 1250c+1250).)

    feature = np.asarray(feature, dtype=np.float32)
    CTC = np.asarray(CTC, dtype=np.float32)

    sel3 = np.zeros((3, 30), dtype=np.float32)
    for r in range(3):
        sel3[r, r * 10:(r + 1) * 10] = 1.0
    sel11 = np.zeros((K + 1, (K + 1) * C), dtype=np.float32)
    for r in range(K + 1):
        sel11[r, r * C:(r + 1) * C] = 1.0

    in_maps = []
    for k in range(NC):
        r0, r1 = k * NSH, (k + 1) * NSH
        dloc = dinv[r0:r1]
        dinvs = np.stack([dloc, -dloc, -2.0 * dloc]).astype(np.float32)
        in_maps.append({
            "a8": np.ascontiguousarray(a8[:, r0:r1]),
            "featT": np.ascontiguousarray(feature[r0:r1].T),
            "ctct": np.ascontiguousarray(CTC[r0:r1].astype(np.float16).T),
            "w1": np.asarray(W1, dtype=np.float32),
            "b1": np.asarray(b1, dtype=np.float32).reshape(HID, 1),
            "w2": np.asarray(W2, dtype=np.float32),
            "b2": np.asarray(b2, dtype=np.float32).reshape(C, 1),
            "wp": np.ascontiguousarray(np.asarray(Wp, dtype=np.float32).transpose(1, 0, 2).reshape(C, (K + 1) * RANK)).astype(np.float16),
            "bp": np.ascontiguousarray(np.asarray(bp, dtype=np.float32).T),
            "gam": (np.asarray(gamma, dtype=np.float32) / RANK).astype(np.float16),
            "dinvs": dinvs,
            "sel3": sel3,
            "sel11": sel11,
        })
    return in_maps


def kernel(feature, edges, CTC, W1, b1, W2, b2, gamma, Wp, bp):
    from concourse.bass_utils import run_bass_kernel_spmd

    if "nc" not in _CACHE:
        _CACHE["nc"] = _build_program()
    nc = _CACHE["nc"]

    in_maps = _host_prep(feature, edges, CTC, W1, b1, W2, b2, gamma, Wp, bp)
    trace = bool(os.environ.get("GNN_TRACE"))
    res = run_bass_kernel_spmd(nc, in_maps, list(range(NC)), trace=trace)
    _CACHE["last_result"] = res
    out = np.concatenate([res.results[k]["out"] for k in range(NC)], axis=0)
    return out.astype(np.float32)



# revision 19
# speedup vs baseline: 1.3396x; 1.3396x over previous
"""CPFGNN Trainium2 kernel: 8-core SPMD Bass implementation (v2).

Math (exact simplifications of the reference):
  - lam = 2.0 always (w_off <= 0), so diag = 0 and prop(t) is a pure
    edge scatter-add: prop(t) = -D^-1/2 A^T D^-1/2 t, with A the
    (multi-)adjacency count matrix excluding self-loops and deg = out-degree.
  - The 11 CTC @ e_k matvecs batch into one CTC @ E (N x 11) pass.

v2 structure (vs v1):
  - A (fp8 exact counts, [N, NSH] per core) is resident in SBUF for the
    whole kernel: zero per-hop HBM traffic for the hop matmuls.
  - Hop matmuls run in fp8 DoubleRow perf mode: two 128-row j-tiles are
    contracted per instruction (2x PE throughput). Pad rows of the last
    j-tile per core-block are zeroed once so full-128 pairs are safe.
  - Strip-major matmul emission: strip s's PSUM closes while s+1 still
    streams, so the evac + Chebyshev combine + next-u limb split +
    AllGather staging all overlap the tensor burst.
  - The per-hop AllGather (u limbs, fp8, 40KB/core) runs on the gpsimd
    queue with explicit semaphores (no tile_critical, no engine drain);
    eta_k's small matmuls are emitted after the collective so they fill
    the gather window.
  - Tx history is spilled to DRAM (fp16) and re-read in the tail,
    freeing SBUF for the resident A.
  - MLP + CTC stream in fp16 (validated: end-to-end rel err ~6e-4).
"""
import os
import sys

sys.path.insert(0, "/opt/trn_rl_repo")

import numpy as np
import ml_dtypes
from contextlib import ExitStack

N = 10000
E_EDGES = 320000
F_IN = 500
HID = 64
C = 10
RANK = 3
K = 10
NC = 8
NSH = N // NC              # 1250 nodes per core
LT = (NSH + 127) // 128    # 10 local node tiles (last partial: 98)
LLAST = NSH - 128 * (LT - 1)  # 98
GJT = NC * LT              # 80 global j-tiles
PAIRS = LT // 2            # 5 DoubleRow pairs per core-block
# (col0, width, first local node tile, #tiles)
STRIPS = [(0, 512, 0, 4), (512, 512, 4, 4), (1024, NSH - 1024, 8, 2)]
UW = 80                    # fp8 u row: hi 0:10, mid 32:42, lo 64:74 (32-aligned for psum reads)
EW = 16                    # fp16 e row: 0:11

NP_FP8 = ml_dtypes.float8_e4m3
NP_BF16 = ml_dtypes.bfloat16

_CACHE = {}


def _build_program():
    import concourse.bass as bass
    import concourse.tile as tile
    from concourse import bacc, mybir
    from concourse.masks import make_identity

    dt = mybir.dt
    FP8 = dt.float8e4
    FP16 = dt.float16
    F32 = dt.float32
    AF = mybir.ActivationFunctionType
    ALU = mybir.AluOpType
    DR = mybir.MatmulPerfMode.DoubleRow

    nc = bacc.Bacc("TRN2", target_bir_lowering=False, debug=False, num_devices=NC)

    # ---------------- DRAM I/O ----------------
    a_dram = nc.dram_tensor("a8", [N, NSH], FP8, kind="ExternalInput")
    featT_dram = nc.dram_tensor("featT", [F_IN, NSH], FP16, kind="ExternalInput")
    ctct_dram = nc.dram_tensor("ctct", [N, NSH], FP16, kind="ExternalInput")
    w1_dram = nc.dram_tensor("w1", [F_IN, HID], FP16, kind="ExternalInput")
    b1_dram = nc.dram_tensor("b1", [HID, 1], F32, kind="ExternalInput")
    w2_dram = nc.dram_tensor("w2", [HID, C], FP16, kind="ExternalInput")
    b2_dram = nc.dram_tensor("b2", [C, 1], F32, kind="ExternalInput")
    wp_dram = nc.dram_tensor("wp", [C, (K + 1) * RANK], FP16, kind="ExternalInput")
    bp_dram = nc.dram_tensor("bp", [RANK, K + 1], F32, kind="ExternalInput")
    gam_dram = nc.dram_tensor("gam", [RANK, K + 1], FP16, kind="ExternalInput")
    # rows: 0 = dinv_loc, 1 = -dinv_loc, 2 = -2*dinv_loc   (this core's range)
    dinv_dram = nc.dram_tensor("dinvs", [3, NSH], F32, kind="ExternalInput")
    sel3_dram = nc.dram_tensor("sel3", [3, 30], F32, kind="ExternalInput")
    sel11_dram = nc.dram_tensor("sel11", [K + 1, (K + 1) * C], F32, kind="ExternalInput")
    out_dram = nc.dram_tensor("out", [NSH, C], F32, kind="ExternalOutput")
    hist_dram = nc.dram_tensor("hist", [K + 1, C, NSH], FP16)
    DEBUG = bool(os.environ.get("GNN_DEBUG"))
    if DEBUG:
        dump_tx = nc.dram_tensor("dump_tx", [K + 1, C, NSH], F32, kind="ExternalOutput")
        dump_e = nc.dram_tensor("dump_e", [K + 1, NSH], F32, kind="ExternalOutput")
        dump_eta = nc.dram_tensor("dump_eta", [K + 1, NSH], F32, kind="ExternalOutput")
        dump_hid = nc.dram_tensor("dump_hid", [C, NSH], F32, kind="ExternalOutput")

    RG = [list(range(NC))]

    with ExitStack() as ctx:
        tc = ctx.enter_context(tile.TileContext(nc))
        const = ctx.enter_context(tc.tile_pool(name="const", bufs=1))
        small = ctx.enter_context(tc.tile_pool(name="small", bufs=3))
        stream = ctx.enter_context(tc.tile_pool(name="stream", bufs=3))
        dram = ctx.enter_context(tc.tile_pool(name="dram", bufs=2, space="DRAM"))

        # ------------- resident tensors -------------
        A8 = const.tile([128, NC, LT, NSH], FP8, tag="A8")
        u_stat = const.tile([128, NC, LT, UW], FP8, tag="u_stat")
        u_loc8 = const.tile([128, LT, UW], FP8, tag="u_loc8")
        e_stat = const.tile([128, NC, LT, EW], FP16, tag="e_stat")
        e_loc = const.tile([128, LT, EW], FP16, tag="e_loc")

        w1s = const.tile([128, 4, HID], FP16, tag="w1")
        nc.sync.dma_start(
            w1s[:, 0:3, :], w1_dram[0:384, :].rearrange("(t p) c -> p t c", p=128)
        )
        nc.sync.dma_start(w1s[0:F_IN - 384, 3, :], w1_dram[384:F_IN, :])
        b1s = const.tile([HID, 1], F32, tag="b1")
        nc.sync.dma_start(b1s[:], b1_dram[:])
        w2s = const.tile([HID, C], FP16, tag="w2")
        nc.sync.dma_start(w2s[:], w2_dram[:])
        b2s = const.tile([C, 1], F32, tag="b2")
        nc.sync.dma_start(b2s[:], b2_dram[:])
        wps = const.tile([C, (K + 1) * RANK], FP16, tag="wp")
        nc.sync.dma_start(wps[:], wp_dram[:])
        bps = const.tile([RANK, K + 1], F32, tag="bp")
        nc.sync.dma_start(bps[:], bp_dram[:])
        gams = const.tile([RANK, K + 1], FP16, tag="gam")
        nc.sync.dma_start(gams[:], gam_dram[:])
        sel11s = const.tile([K + 1, (K + 1) * C], F32, tag="sel11")
        nc.sync.dma_start(sel11s[:], sel11_dram[:])
        ident = const.tile([128, 128], F32, tag="ident")
        make_identity(nc, ident[:])

        st = [const.tile([C, NSH], F32, tag=f"st{i}", name=f"state{i}")
              for i in range(3)]
        eT = const.tile([K + 1, NSH], F32, tag="eT")

        # zero DoubleRow pad rows (tile LT-1 has only LLAST valid rows).
        # Engine APs must start at a 32-aligned partition, so zero from 96;
        # rows 96..97 are rewritten by the A DMA / per-hop limb writes.
        nc.vector.memset(A8[96:128, :, LT - 1, :], 0.0)
        nc.vector.memset(u_loc8[:], 0.0)
        nc.vector.memset(e_loc[96:128, LT - 1, :], 0.0)

        # A load: per core-block, 9 aligned tiles + 98-row tail
        for cg in range(NC):
            r0 = cg * NSH
            nc.sync.dma_start(
                A8[:, cg, 0:LT - 1, :],
                a_dram[r0:r0 + 128 * (LT - 1), :].rearrange("(t p) c -> p t c", p=128),
            )
            nc.sync.dma_start(A8[0:LLAST, cg, LT - 1, :],
                              a_dram[r0 + 128 * (LT - 1):r0 + NSH, :])

        # dB[r] = broadcast of dinvs row r to C partitions; MLP-only tensors
        # (dinvs, sel3, x1T) live in a scoped pool freed before the hops.
        dB = [const.tile([C, NSH], F32, tag=f"dB{r}", name=f"dB{r}") for r in range(3)]
        KT = [(0, 128), (128, 128), (256, 128), (384, F_IN - 384)]
        h16_of = {}
        with tc.tile_pool(name="tmp0", bufs=1) as tmp0, \
             tc.tile_pool(name="psmlp", bufs=3, space="PSUM") as psmlp:
            dinvs = tmp0.tile([3, NSH], F32, tag="dinvs")
            nc.sync.dma_start(dinvs[:], dinv_dram[:])
            sel3s = tmp0.tile([3, 30], F32, tag="sel3")
            nc.sync.dma_start(sel3s[:], sel3_dram[:])
            x1T = tmp0.tile([HID, NSH], FP16, tag="x1T")
            for r in range(3):
                for s0, sw, _, _ in STRIPS:
                    psd = psmlp.tile([C, 512], F32, space="PSUM", tag="ps2",
                                     name=f"psd{r}")
                    nc.tensor.matmul(psd[:, 0:sw], sel3s[:, r * 10:(r + 1) * 10],
                                     dinvs[:, s0:s0 + sw], start=True, stop=True)
                    nc.vector.tensor_copy(dB[r][:, s0:s0 + sw], psd[:, 0:sw])

            # ---------------- MLP (ki-major so 3 stream bufs suffice) --------
            pss1 = [psmlp.tile([HID, 512], F32, space="PSUM", tag=f"psA{si}",
                               name=f"psA{si}", bufs=1) for si in range(3)]
            for ki, (k0, kw) in enumerate(KT):
                ft = stream.tile([128, NSH], FP16, tag="mv", name=f"ft{ki}")
                nc.sync.dma_start(ft[0:kw, :], featT_dram[k0:k0 + kw, :])
                for si, (s0, sw, _, _) in enumerate(STRIPS):
                    nc.tensor.matmul(
                        pss1[si][:, 0:sw], w1s[0:kw, ki, :], ft[0:kw, s0:s0 + sw],
                        start=(ki == 0), stop=(ki == 3),
                    )
            for si, (s0, sw, _, _) in enumerate(STRIPS):
                nc.scalar.activation(x1T[:, s0:s0 + sw], pss1[si][:, 0:sw], AF.Relu,
                                     bias=b1s[:], scale=1.0)
            for si, (s0, sw, _, _) in enumerate(STRIPS):
                ps2 = psmlp.tile([C, 512], F32, space="PSUM", tag="ps2", name="psB")
                nc.tensor.matmul(ps2[:, 0:sw], w2s[:], x1T[:, s0:s0 + sw],
                                 start=True, stop=True)
                nc.scalar.activation(st[0][:, s0:s0 + sw], ps2[:, 0:sw], AF.Identity,
                                     bias=b2s[:], scale=1.0)
                h16 = small.tile([C, 512], FP16, tag="h16", name=f"h16_0_{si}")
                nc.scalar.activation(h16[:, 0:sw], st[0][:, s0:s0 + sw], AF.Copy)
                nc.sync.dma_start(hist_dram[0, :, s0:s0 + sw], h16[:, 0:sw])
                h16_of[si] = h16
        if DEBUG:
            nc.sync.dma_start(dump_tx[0], st[0][:])

        # -------- hop-phase pools: 6 strip psum banks + 2 aux banks --------
        with tc.tile_pool(name="psH", bufs=2, space="PSUM") as psH, \
             tc.tile_pool(name="psX", bufs=2, space="PSUM") as psX:

            def prep_strip(si, st_cur, tag):
                """u = dinv*t for one strip -> fp8 limbs in u_loc8 -> stage to DRAM."""
                s0, sw, t0, nt = STRIPS[si]
                u_s = small.tile([C, 512], F32, tag="u_s", name=f"u_{tag}_{si}", bufs=2)
                nc.vector.tensor_tensor(out=u_s[:, 0:sw], in0=st_cur[:, s0:s0 + sw],
                                        in1=dB[0][:, s0:s0 + sw], op=ALU.mult)
                for ti in range(nt):
                    t = t0 + ti
                    pw = 128 if t < LT - 1 else LLAST
                    psT = psX.tile([128, 512], F32, space="PSUM", tag="aux",
                                   name=f"psT_{tag}_{t}")
                    nc.tensor.transpose(psT[0:pw, 0:C], u_s[:, ti * 128:ti * 128 + pw],
                                        ident[0:C, 0:C])
                    nc.scalar.activation(u_loc8[0:pw, t, 0:10], psT[0:pw, 0:C], AF.Copy)
                    hif = small.tile([128, C], F32, tag="hif", name=f"hif_{tag}_{t}")
                    nc.scalar.activation(hif[0:pw, :], u_loc8[0:pw, t, 0:10], AF.Copy)
                    r1 = small.tile([128, C], F32, tag="r1", name=f"r1_{tag}_{t}")
                    nc.vector.tensor_tensor(out=r1[0:pw, :], in0=psT[0:pw, 0:C],
                                            in1=hif[0:pw, :], op=ALU.subtract)
                    nc.scalar.activation(u_loc8[0:pw, t, 32:42], r1[0:pw, :],
                                         AF.Copy, scale=64.0)
                    midf = small.tile([128, C], F32, tag="midf", name=f"midf_{tag}_{t}")
                    nc.scalar.activation(midf[0:pw, :], u_loc8[0:pw, t, 32:42],
                                         AF.Copy, scale=1.0 / 64.0)
                    r2 = small.tile([128, C], F32, tag="r2", name=f"r2_{tag}_{t}")
                    nc.vector.tensor_tensor(out=r2[0:pw, :], in0=r1[0:pw, :],
                                            in1=midf[0:pw, :], op=ALU.subtract)
                    nc.scalar.activation(u_loc8[0:pw, t, 64:74], r2[0:pw, :],
                                         AF.Copy, scale=4096.0)

            def stage_strip(si, agu_in):
                _, _, t0, nt = STRIPS[si]
                nc.sync.dma_start(agu_in[:, t0:t0 + nt, :],
                                  u_loc8[:, t0:t0 + nt, :])

            def launch_ag_u(agu_in, k):
                agu_out = dram.tile([NC, 128, LT, UW], FP8, tag="agout",
                                    name=f"agout{k}", addr_space="Shared")
                nc.gpsimd.collective_compute(
                    "AllGather", ALU.bypass, replica_groups=RG,
                    ins=[agu_in[:]], outs=[agu_out[:]],
                )
                nc.gpsimd.dma_start(
                    out=u_stat[:],
                    in_=agu_out[:].rearrange("c p t x -> p c t x"),
                )

            def compute_eta(k, h16s):
                """eT[k] = tanh(Txk @ Wp[k] + bp[k]) @ (gamma[:,k]/3)."""
                pshs, htas = [], []
                for si, (s0, sw, _, _) in enumerate(STRIPS):
                    psh = psX.tile([128, 512], F32, space="PSUM", tag="aux",
                                   name=f"psh{k}_{si}")
                    nc.tensor.matmul(psh[0:RANK, 0:sw],
                                     wps[:, k * RANK:(k + 1) * RANK],
                                     h16s[si][:, 0:sw], start=True, stop=True)
                    hta = small.tile([RANK, 512], FP16, tag="hta",
                                     name=f"hta{k}_{si}")
                    nc.scalar.activation(hta[:, 0:sw], psh[0:RANK, 0:sw], AF.Tanh,
                                         bias=bps[:, k:k + 1], scale=1.0)
                    pshs.append(psh); htas.append(hta)
                eRow = small.tile([1, NSH], F32, tag="eRow", name=f"eRow{k}", bufs=1)
                for si, (s0, sw, _, _) in enumerate(STRIPS):
                    pse2 = psX.tile([128, 512], F32, space="PSUM", tag="aux",
                                    name=f"pse2{k}_{si}")
                    nc.tensor.matmul(pse2[0:1, 0:sw], gams[:, k:k + 1],
                                     htas[si][:, 0:sw], start=True, stop=True)
                    nc.vector.tensor_copy(eRow[:, s0:s0 + sw], pse2[0:1, 0:sw])
                nc.sync.dma_start(eT[k:k + 1, :], eRow[:])

            # ---------------- prologue ----------------
            agu_in = dram.tile([128, LT, UW], FP8, tag="agin", name="agin0")
            for si in range(3):
                prep_strip(si, st[0], "p")
                stage_strip(si, agu_in)
            launch_ag_u(agu_in, 0)
            compute_eta(0, h16_of)

            # ---------------- hops ----------------
            cur_i, prev_i, free_i = 0, None, 1
            for k in range(1, K + 1):
                # strip-major DoubleRow matmul burst (waits on u_stat DMA)
                pss = []
                for si, (s0, sw, _, _) in enumerate(STRIPS):
                    ps = psH.tile([74, 512], F32, space="PSUM", tag=f"s{si}",
                                  name=f"hop{k}s{si}")
                    pss.append(ps)
                for si, (s0, sw, _, _) in enumerate(STRIPS):
                    for cg in range(NC):
                        for i in range(PAIRS):
                            jg = cg * PAIRS + i
                            nc.tensor.matmul(
                                pss[si][:, 0:sw],
                                u_stat[:, cg, 2 * i:2 * i + 2, 0:74],
                                A8[:, cg, 2 * i:2 * i + 2, s0:s0 + sw],
                                start=(jg == 0), stop=(jg == NC * PAIRS - 1),
                                perf_mode=DR,
                            )
                # per-strip: evac + Chebyshev combine + next-u prep
                scale_rows = dB[1] if k == 1 else dB[2]
                nxt = st[free_i]
                if k < K:
                    agu_in = dram.tile([128, LT, UW], FP8, tag="agin",
                                       name=f"agin{k}")
                h16s = {}
                for si, (s0, sw, t0, nt) in enumerate(STRIPS):
                    ps = pss[si]
                    m1 = small.tile([C, 512], F32, tag="ev1", name=f"m1_{k}_{si}", bufs=2)
                    nc.scalar.activation(m1[:, 0:sw], ps[32:42, 0:sw], AF.Copy,
                                         scale=1.0 / 64.0)
                    l1 = small.tile([C, 512], F32, tag="ev2", name=f"l1_{k}_{si}", bufs=2)
                    nc.scalar.activation(l1[:, 0:sw], ps[64:74, 0:sw], AF.Copy,
                                         scale=1.0 / 4096.0)
                    a1 = small.tile([C, 512], F32, tag="ev1", name=f"a1_{k}_{si}", bufs=2)
                    nc.vector.tensor_tensor(out=a1[:, 0:sw], in0=ps[0:10, 0:sw],
                                            in1=m1[:, 0:sw], op=ALU.add)
                    a2 = small.tile([C, 512], F32, tag="ev2", name=f"a2_{k}_{si}", bufs=2)
                    nc.vector.tensor_tensor(out=a2[:, 0:sw], in0=a1[:, 0:sw],
                                            in1=l1[:, 0:sw], op=ALU.add)
                    if k == 1:
                        nc.vector.tensor_tensor(out=nxt[:, s0:s0 + sw],
                                                in0=a2[:, 0:sw],
                                                in1=scale_rows[:, s0:s0 + sw],
                                                op=ALU.mult)
                    else:
                        t3 = small.tile([C, 512], F32, tag="ev1",
                                        name=f"t3_{k}_{si}", bufs=2)
                        nc.vector.tensor_tensor(out=t3[:, 0:sw], in0=a2[:, 0:sw],
                                                in1=scale_rows[:, s0:s0 + sw],
                                                op=ALU.mult)
                        nc.vector.tensor_tensor(out=nxt[:, s0:s0 + sw],
                                                in0=t3[:, 0:sw],
                                                in1=st[prev_i][:, s0:s0 + sw],
                                                op=ALU.subtract)
                    h16 = small.tile([C, 512], FP16, tag="h16", name=f"h16_{k}_{si}")
                    nc.scalar.activation(h16[:, 0:sw], nxt[:, s0:s0 + sw], AF.Copy)
                    nc.sync.dma_start(hist_dram[k, :, s0:s0 + sw], h16[:, 0:sw])
                    h16s[si] = h16
                    if k < K:
                        prep_strip(si, nxt, f"h{k}")
                        stage_strip(si, agu_in)
                if DEBUG:
                    nc.sync.dma_start(dump_tx[k], nxt[:])
                prev_i, cur_i = cur_i, free_i
                free_i = 3 - cur_i - prev_i
                if k < K:
                    launch_ag_u(agu_in, k)
                compute_eta(k, h16s)

        # ---------------- E allgather ----------------
        with tc.tile_pool(name="psT2", bufs=2, space="PSUM") as psT2, \
             tc.tile_pool(name="tailp", bufs=1) as tailp:
            etaS = tailp.tile([K + 1, NSH], F32, tag="etaS")
            hidT = tailp.tile([C, NSH], F32, tag="hidT")
            for t in range(LT):
                pw = 128 if t < LT - 1 else LLAST
                psE = psT2.tile([128, 512], F32, space="PSUM", tag="aux2",
                                name=f"psE{t}")
                nc.tensor.transpose(psE[0:pw, 0:K + 1],
                                    eT[:, t * 128:t * 128 + pw],
                                    ident[0:K + 1, 0:K + 1])
                nc.scalar.activation(e_loc[0:pw, t, 0:K + 1], psE[0:pw, 0:K + 1],
                                     AF.Copy)
            if DEBUG:
                nc.sync.dma_start(dump_e[:], eT[:])
            age_in = dram.tile([128, LT, EW], FP16, tag="agein", name="agein")
            age_out = dram.tile([NC, 128, LT, EW], FP16, tag="ageout",
                                name="ageout", addr_space="Shared")
            nc.sync.dma_start(age_in[:], e_loc[:])
            nc.gpsimd.collective_compute(
                "AllGather", ALU.bypass, replica_groups=RG,
                ins=[age_in[:]], outs=[age_out[:]],
            )
            nc.gpsimd.dma_start(
                out=e_stat[:],
                in_=age_out[:].rearrange("c p t x -> p c t x"),
            )

            # ---------------- CTC @ E ----------------
            with tc.tile_pool(name="psC", bufs=1, space="PSUM") as psC:
                pcs = [psC.tile([K + 1, 512], F32, space="PSUM", tag=f"c{si}",
                                name=f"ctc{si}") for si in range(3)]
                for jg in range(GJT):
                    cg, t = jg // LT, jg % LT
                    kw = 128 if t < LT - 1 else LLAST
                    row0 = cg * NSH + t * 128
                    cj = stream.tile([128, NSH], FP16, tag="mv", name=f"cj{jg}")
                    nc.sync.dma_start(cj[0:kw, :], ctct_dram[row0:row0 + kw, :])
                    for si, (s0, sw, _, _) in enumerate(STRIPS):
                        nc.tensor.matmul(
                            pcs[si][:, 0:sw], e_stat[0:kw, cg, t, 0:K + 1],
                            cj[0:kw, s0:s0 + sw],
                            start=(jg == 0), stop=(jg == GJT - 1),
                        )
                for si, (s0, sw, _, _) in enumerate(STRIPS):
                    nc.vector.tensor_copy(etaS[:, s0:s0 + sw], pcs[si][:, 0:sw])
            if DEBUG:
                nc.sync.dma_start(dump_eta[:], etaS[:])

            # ---------------- hidden = sum_k Txk * eta_k ----------------
            for si, (s0, sw, _, _) in enumerate(STRIPS):
                accV = tailp.tile([C, 512], F32, tag="accV", name=f"accV{si}",
                                  bufs=2)
                accP = tailp.tile([C, 512], F32, tag="accP", name=f"accP{si}",
                                  bufs=2)
                for k in range(K + 1):
                    hk = tailp.tile([C, 512], FP16, tag="hk",
                                    name=f"hk{si}_{k}", bufs=4)
                    nc.sync.dma_start(hk[:, 0:sw], hist_dram[k, :, s0:s0 + sw])
                    psr = psT2.tile([128, 512], F32, space="PSUM", tag="aux2",
                                    name=f"psr{si}_{k}")
                    nc.tensor.matmul(psr[0:C, 0:sw], sel11s[:, k * C:(k + 1) * C],
                                     etaS[:, s0:s0 + sw], start=True, stop=True)
                    pr = tailp.tile([C, 512], F32, tag="prsb",
                                    name=f"pr{si}_{k}", bufs=2)
                    nc.scalar.activation(pr[:, 0:sw], psr[0:C, 0:sw], AF.Copy)
                    eng = nc.vector if (k % 2 == 0) else nc.gpsimd
                    acc = accV if (k % 2 == 0) else accP
                    tmp = tailp.tile([C, 512], F32, tag="htmp",
                                     name=f"htmp{si}_{k}", bufs=2)
                    eng.tensor_tensor(out=tmp[:, 0:sw],
                                      in0=hk[:, 0:sw],
                                      in1=pr[:, 0:sw], op=ALU.mult)
                    if k < 2:
                        eng.tensor_copy(acc[:, 0:sw], tmp[:, 0:sw])
                    else:
                        eng.tensor_tensor(out=acc[:, 0:sw], in0=acc[:, 0:sw],
                                          in1=tmp[:, 0:sw], op=ALU.add)
                nc.vector.tensor_tensor(out=hidT[:, s0:s0 + sw], in0=accV[:, 0:sw],
                                        in1=accP[:, 0:sw], op=ALU.add)
            if DEBUG:
                nc.sync.dma_start(dump_hid[:], hidT[:])

            # ---------------- log_softmax + out ----------------
            for t in range(LT):
                pw = 128 if t < LT - 1 else LLAST
                psS = psT2.tile([128, 512], F32, space="PSUM", tag="aux2",
                                name=f"psS{t}")
                nc.tensor.transpose(psS[0:pw, 0:C], hidT[:, t * 128:t * 128 + pw],
                                    ident[0:C, 0:C])
                h = small.tile([128, C], F32, tag="hrow", name=f"hrow{t}")
                nc.vector.tensor_copy(h[0:pw, :], psS[0:pw, 0:C])
                mx = small.tile([128, 1], F32, tag="mx", name=f"mx{t}")
                nc.vector.tensor_reduce(mx[0:pw, :], h[0:pw, :],
                                        axis=mybir.AxisListType.X, op=ALU.max)
                sh = small.tile([128, C], F32, tag="sh", name=f"sh{t}")
                nc.vector.tensor_scalar_sub(sh[0:pw, :], h[0:pw, :], mx[0:pw, :])
                ex = small.tile([128, C], F32, tag="ex", name=f"ex{t}")
                sm = small.tile([128, 1], F32, tag="sm", name=f"sm{t}")
                nc.scalar.activation(ex[0:pw, :], sh[0:pw, :], AF.Exp,
                                     accum_out=sm[0:pw, :])
                ls = small.tile([128, 1], F32, tag="ls", name=f"ls{t}")
                nc.scalar.activation(ls[0:pw, :], sm[0:pw, :], AF.Ln)
                o = small.tile([128, C], F32, tag="o", name=f"o{t}")
                nc.vector.tensor_scalar_sub(o[0:pw, :], sh[0:pw, :], ls[0:pw, :])
                nc.sync.dma_start(out_dram[t * 128:t * 128 + pw, :], o[0:pw, :])

    nc.compile()
    return nc


def _host_prep(feature, edges, CTC, W1, b1, W2, b2, gamma, Wp, bp):
    src = np.asarray(edges[0], dtype=np.int64)
    dst = np.asarray(edges[1], dtype=np.int64)
    nonself = src != dst
    s, d = src[nonself], dst[nonself]

    deg = np.bincount(s, minlength=N).astype(np.float64)
    dinv = np.where(deg > 0, 1.0 / np.sqrt(np.maximum(deg, 1e-30)), 0.0).astype(np.float32)

    counts = np.zeros((N, N), dtype=np.uint8)
    np.add.at(counts, (s, d), 1)
    lut = np.arange(256).astype(NP_FP8)
    a8 = lut[counts]          # [N, N] fp8, exact small ints

    feature = np.asarray(feature, dtype=np.float32)
    CTC = np.asarray(CTC, dtype=np.float32)

    sel3 = np.zeros((3, 30), dtype=np.float32)
    for r in range(3):
        sel3[r, r * 10:(r + 1) * 10] = 1.0
    sel11 = np.zeros((K + 1, (K + 1) * C), dtype=np.float32)
    for r in range(K + 1):
        sel11[r, r * C:(r + 1) * C] = 1.0

    in_maps = []
    for k in range(NC):
        r0, r1 = k * NSH, (k + 1) * NSH
        dloc = dinv[r0:r1]
        dinvs = np.stack([dloc, -dloc, -2.0 * dloc]).astype(np.float32)
        in_maps.append({
            "a8": np.ascontiguousarray(a8[:, r0:r1]),
            "featT": np.ascontiguousarray(feature[r0:r1].T.astype(np.float16)),
            "ctct": np.ascontiguousarray(CTC[r0:r1].astype(np.float16).T),
            "w1": np.asarray(W1, dtype=np.float16),
            "b1": np.asarray(b1, dtype=np.float32).reshape(HID, 1),
            "w2": np.asarray(W2, dtype=np.float16),
            "b2": np.asarray(b2, dtype=np.float32).reshape(C, 1),
            "wp": np.ascontiguousarray(np.asarray(Wp, dtype=np.float32).transpose(1, 0, 2).reshape(C, (K + 1) * RANK)).astype(np.float16),
            "bp": np.ascontiguousarray(np.asarray(bp, dtype=np.float32).T),
            "gam": (np.asarray(gamma, dtype=np.float32) / RANK).astype(np.float16),
            "dinvs": dinvs,
            "sel3": sel3,
            "sel11": sel11,
        })
    return in_maps


def kernel(feature, edges, CTC, W1, b1, W2, b2, gamma, Wp, bp):
    from concourse.bass_utils import run_bass_kernel_spmd

    if "nc" not in _CACHE:
        _CACHE["nc"] = _build_program()
    nc = _CACHE["nc"]

    in_maps = _host_prep(feature, edges, CTC, W1, b1, W2, b2, gamma, Wp, bp)
    trace = bool(os.environ.get("GNN_TRACE"))
    res = run_bass_kernel_spmd(nc, in_maps, list(range(NC)), trace=trace)
    _CACHE["last_result"] = res
    out = np.concatenate([res.results[k]["out"] for k in range(NC)], axis=0)
    return out.astype(np.float32)


# revision 21
# speedup vs baseline: 1.8528x; 1.3831x over previous
"""CPFGNN Trainium2 kernel: 8-core SPMD Bass implementation (v2).

Math (exact simplifications of the reference):
  - lam = 2.0 always (w_off <= 0), so diag = 0 and prop(t) is a pure
    edge scatter-add: prop(t) = -D^-1/2 A^T D^-1/2 t, with A the
    (multi-)adjacency count matrix excluding self-loops and deg = out-degree.
  - The 11 CTC @ e_k matvecs batch into one CTC @ E (N x 11) pass.

v2 structure (vs v1):
  - A (fp8 exact counts, [N, NSH] per core) is resident in SBUF for the
    whole kernel: zero per-hop HBM traffic for the hop matmuls.
  - Hop matmuls run in fp8 DoubleRow perf mode: two 128-row j-tiles are
    contracted per instruction (2x PE throughput). Pad rows of the last
    j-tile per core-block are zeroed once so full-128 pairs are safe.
  - Strip-major matmul emission: strip s's PSUM closes while s+1 still
    streams, so the evac + Chebyshev combine + next-u limb split +
    AllGather staging all overlap the tensor burst.
  - The per-hop AllGather (u limbs, fp8, 40KB/core) runs on the gpsimd
    queue with explicit semaphores (no tile_critical, no engine drain);
    eta_k's small matmuls are emitted after the collective so they fill
    the gather window.
  - Tx history is spilled to DRAM (fp16) and re-read in the tail,
    freeing SBUF for the resident A.
  - MLP + CTC stream in fp16 (validated: end-to-end rel err ~6e-4).
"""
import os
import sys

sys.path.insert(0, "/opt/trn_rl_repo")

import numpy as np
import ml_dtypes
from contextlib import ExitStack

N = 10000
E_EDGES = 320000
F_IN = 500
HID = 64
C = 10
RANK = 3
K = 10
NC = 8
NSH = N // NC              # 1250 nodes per core
LT = (NSH + 127) // 128    # 10 local node tiles (last partial: 98)
LLAST = NSH - 128 * (LT - 1)  # 98
GJT = NC * LT              # 80 global j-tiles
PAIRS = LT // 2            # 5 DoubleRow pairs per core-block
# (col0, width, first local node tile, #tiles)
STRIPS = [(0, 512, 0, 4), (512, 512, 4, 4), (1024, NSH - 1024, 8, 2)]
UW = 48                    # fp8 u row: hi 0:10, mid 32:42 (32-aligned for psum reads)
EW = 16                    # fp16 e row: 0:11

NP_FP8 = ml_dtypes.float8_e4m3
NP_BF16 = ml_dtypes.bfloat16

_CACHE = {}


def _build_program():
    import concourse.bass as bass
    import concourse.tile as tile
    from concourse import bacc, mybir
    from concourse.masks import make_identity

    dt = mybir.dt
    FP8 = dt.float8e4
    FP16 = dt.float16
    F32 = dt.float32
    AF = mybir.ActivationFunctionType
    ALU = mybir.AluOpType
    DR = mybir.MatmulPerfMode.DoubleRow

    nc = bacc.Bacc("TRN2", target_bir_lowering=False, debug=False, num_devices=NC)

    # ---------------- DRAM I/O ----------------
    a_dram = nc.dram_tensor("a8", [N, NSH], FP8, kind="ExternalInput")
    featT_dram = nc.dram_tensor("featT", [F_IN, NSH], FP16, kind="ExternalInput")
    ctct_dram = nc.dram_tensor("ctct", [N, NSH], FP16, kind="ExternalInput")
    w1_dram = nc.dram_tensor("w1", [F_IN, HID], FP16, kind="ExternalInput")
    b1_dram = nc.dram_tensor("b1", [HID, 1], F32, kind="ExternalInput")
    w2_dram = nc.dram_tensor("w2", [HID, C], FP16, kind="ExternalInput")
    b2_dram = nc.dram_tensor("b2", [C, 1], F32, kind="ExternalInput")
    wp_dram = nc.dram_tensor("wp", [C, (K + 1) * RANK], FP16, kind="ExternalInput")
    bp_dram = nc.dram_tensor("bp", [RANK, K + 1], F32, kind="ExternalInput")
    gam_dram = nc.dram_tensor("gam", [RANK, K + 1], FP16, kind="ExternalInput")
    # rows: 0 = dinv_loc, 1 = -dinv_loc, 2 = -2*dinv_loc   (this core's range)
    dinv_dram = nc.dram_tensor("dinvs", [3, NSH], F32, kind="ExternalInput")
    sel3_dram = nc.dram_tensor("sel3", [3, 30], F32, kind="ExternalInput")
    sel11_dram = nc.dram_tensor("sel11", [K + 1, (K + 1) * C], F32, kind="ExternalInput")
    sel11t_dram = nc.dram_tensor("sel11t", [(K + 1) * C, C], FP16, kind="ExternalInput")
    out_dram = nc.dram_tensor("out", [NSH, C], F32, kind="ExternalOutput")
    hist_dram = nc.dram_tensor("hist", [K + 1, C, NSH], FP16)
    DEBUG = bool(os.environ.get("GNN_DEBUG"))
    if DEBUG:
        dump_tx = nc.dram_tensor("dump_tx", [K + 1, C, NSH], F32, kind="ExternalOutput")
        dump_e = nc.dram_tensor("dump_e", [K + 1, NSH], F32, kind="ExternalOutput")
        dump_eta = nc.dram_tensor("dump_eta", [K + 1, NSH], F32, kind="ExternalOutput")
        dump_hid = nc.dram_tensor("dump_hid", [C, NSH], F32, kind="ExternalOutput")

    RG = [list(range(NC))]

    with ExitStack() as ctx:
        tc = ctx.enter_context(tile.TileContext(nc))
        const = ctx.enter_context(tc.tile_pool(name="const", bufs=1))
        small = ctx.enter_context(tc.tile_pool(name="small", bufs=3))
        stream = ctx.enter_context(tc.tile_pool(name="stream", bufs=8))
        dram = ctx.enter_context(tc.tile_pool(name="dram", bufs=2, space="DRAM"))

        # ------------- resident tensors -------------
        A8 = const.tile([128, NC, LT, NSH], FP8, tag="A8")
        u_stat = const.tile([128, NC, LT, UW], FP8, tag="u_stat")
        u_loc8 = const.tile([128, LT, UW], FP8, tag="u_loc8")
        e_stat = const.tile([128, NC, LT, EW], FP16, tag="e_stat")
        e_loc = const.tile([128, LT, EW], FP16, tag="e_loc")

        w1s = const.tile([128, 4, HID], FP16, tag="w1")
        nc.sync.dma_start(
            w1s[:, 0:3, :], w1_dram[0:384, :].rearrange("(t p) c -> p t c", p=128)
        )
        nc.sync.dma_start(w1s[0:F_IN - 384, 3, :], w1_dram[384:F_IN, :])
        b1s = const.tile([HID, 1], F32, tag="b1")
        nc.sync.dma_start(b1s[:], b1_dram[:])
        w2s = const.tile([HID, C], FP16, tag="w2")
        nc.sync.dma_start(w2s[:], w2_dram[:])
        b2s = const.tile([C, 1], F32, tag="b2")
        nc.sync.dma_start(b2s[:], b2_dram[:])
        wps = const.tile([C, (K + 1) * RANK], FP16, tag="wp")
        nc.sync.dma_start(wps[:], wp_dram[:])
        bps = const.tile([RANK, K + 1], F32, tag="bp")
        nc.sync.dma_start(bps[:], bp_dram[:])
        gams = const.tile([RANK, K + 1], FP16, tag="gam")
        nc.sync.dma_start(gams[:], gam_dram[:])
        sel11s = const.tile([K + 1, (K + 1) * C], F32, tag="sel11")
        nc.sync.dma_start(sel11s[:], sel11_dram[:])
        sel11Ts = const.tile([(K + 1) * C, C], FP16, tag="sel11t")
        nc.sync.dma_start(sel11Ts[:], sel11t_dram[:])
        ident = const.tile([128, 128], F32, tag="ident")
        make_identity(nc, ident[:])

        st = [const.tile([C, NSH], F32, tag=f"st{i}", name=f"state{i}")
              for i in range(3)]
        eT = const.tile([K + 1, NSH], F32, tag="eT")

        # zero DoubleRow pad rows (tile LT-1 has only LLAST valid rows).
        # Engine APs must start at a 32-aligned partition, so zero from 96;
        # rows 96..97 are rewritten by the A DMA / per-hop limb writes.
        nc.vector.memset(A8[96:128, :, LT - 1, :], 0.0)
        nc.vector.memset(u_loc8[:], 0.0)
        nc.vector.memset(e_loc[96:128, LT - 1, :], 0.0)

        # A load: per core-block, 9 aligned tiles + 98-row tail. Issued on
        # the gpsimd queue (idle until the first collective) so the sync
        # queue's MLP stream DMAs are not delayed behind 12.5MB of A.
        for cg in range(NC):
            r0 = cg * NSH
            nc.gpsimd.dma_start(
                A8[:, cg, 0:LT - 1, :],
                a_dram[r0:r0 + 128 * (LT - 1), :].rearrange("(t p) c -> p t c", p=128),
            )
            nc.gpsimd.dma_start(A8[0:LLAST, cg, LT - 1, :],
                                a_dram[r0 + 128 * (LT - 1):r0 + NSH, :])

        # dB[r] = broadcast of dinvs row r to C partitions; MLP-only tensors
        # (dinvs, sel3, x1T) live in a scoped pool freed before the hops.
        dB = [const.tile([C, NSH], F32, tag=f"dB{r}", name=f"dB{r}") for r in range(3)]
        KT = [(0, 128), (128, 128), (256, 128), (384, F_IN - 384)]
        h16_of = {}
        with tc.tile_pool(name="tmp0", bufs=1) as tmp0, \
             tc.tile_pool(name="psmlp", bufs=3, space="PSUM") as psmlp:
            dinvs = tmp0.tile([3, NSH], F32, tag="dinvs")
            nc.sync.dma_start(dinvs[:], dinv_dram[:])
            sel3s = tmp0.tile([3, 30], F32, tag="sel3")
            nc.sync.dma_start(sel3s[:], sel3_dram[:])
            x1T = tmp0.tile([HID, NSH], FP16, tag="x1T")
            for r in range(3):
                for s0, sw, _, _ in STRIPS:
                    psd = psmlp.tile([C, 512], F32, space="PSUM", tag="ps2",
                                     name=f"psd{r}")
                    nc.tensor.matmul(psd[:, 0:sw], sel3s[:, r * 10:(r + 1) * 10],
                                     dinvs[:, s0:s0 + sw], start=True, stop=True)
                    nc.vector.tensor_copy(dB[r][:, s0:s0 + sw], psd[:, 0:sw])

            # ---------------- MLP (ki-major so 3 stream bufs suffice) --------
            pss1 = [psmlp.tile([HID, 512], F32, space="PSUM", tag=f"psA{si}",
                               name=f"psA{si}", bufs=1) for si in range(3)]
            for ki, (k0, kw) in enumerate(KT):
                ft = stream.tile([128, NSH], FP16, tag="mv", name=f"ft{ki}")
                nc.sync.dma_start(ft[0:kw, :], featT_dram[k0:k0 + kw, :])
                for si, (s0, sw, _, _) in enumerate(STRIPS):
                    nc.tensor.matmul(
                        pss1[si][:, 0:sw], w1s[0:kw, ki, :], ft[0:kw, s0:s0 + sw],
                        start=(ki == 0), stop=(ki == 3),
                    )
            for si, (s0, sw, _, _) in enumerate(STRIPS):
                nc.scalar.activation(x1T[:, s0:s0 + sw], pss1[si][:, 0:sw], AF.Relu,
                                     bias=b1s[:], scale=1.0)
            for si, (s0, sw, _, _) in enumerate(STRIPS):
                ps2 = psmlp.tile([C, 512], F32, space="PSUM", tag="ps2", name="psB")
                nc.tensor.matmul(ps2[:, 0:sw], w2s[:], x1T[:, s0:s0 + sw],
                                 start=True, stop=True)
                nc.scalar.activation(st[0][:, s0:s0 + sw], ps2[:, 0:sw], AF.Identity,
                                     bias=b2s[:], scale=1.0)
                h16 = small.tile([C, 512], FP16, tag="h16", name=f"h16_0_{si}")
                nc.scalar.activation(h16[:, 0:sw], st[0][:, s0:s0 + sw], AF.Copy)
                nc.sync.dma_start(hist_dram[0, :, s0:s0 + sw], h16[:, 0:sw])
                h16_of[si] = h16
        if DEBUG:
            nc.sync.dma_start(dump_tx[0], st[0][:])

        # -------- hop-phase pools: 6 strip psum banks + 2 aux banks --------
        with tc.tile_pool(name="psH", bufs=2, space="PSUM") as psH, \
             tc.tile_pool(name="psX", bufs=2, space="PSUM") as psX:

            def prep_strip(si, st_cur, tag):
                """u = dinv*t for one strip -> fp8 limbs in u_loc8 -> stage to DRAM."""
                s0, sw, t0, nt = STRIPS[si]
                u_s = small.tile([C, 512], F32, tag="u_s", name=f"u_{tag}_{si}", bufs=2)
                nc.vector.tensor_tensor(out=u_s[:, 0:sw], in0=st_cur[:, s0:s0 + sw],
                                        in1=dB[0][:, s0:s0 + sw], op=ALU.mult)
                for ti in range(nt):
                    t = t0 + ti
                    pw = 128 if t < LT - 1 else LLAST
                    psT = psX.tile([128, 512], F32, space="PSUM", tag="aux",
                                   name=f"psT_{tag}_{t}")
                    nc.tensor.transpose(psT[0:pw, 0:C], u_s[:, ti * 128:ti * 128 + pw],
                                        ident[0:C, 0:C])
                    nc.scalar.activation(u_loc8[0:pw, t, 0:10], psT[0:pw, 0:C], AF.Copy)
                    hif = small.tile([128, C], F32, tag="hif", name=f"hif_{tag}_{t}")
                    nc.scalar.activation(hif[0:pw, :], u_loc8[0:pw, t, 0:10], AF.Copy)
                    r1 = small.tile([128, C], F32, tag="r1", name=f"r1_{tag}_{t}")
                    nc.vector.tensor_tensor(out=r1[0:pw, :], in0=psT[0:pw, 0:C],
                                            in1=hif[0:pw, :], op=ALU.subtract)
                    nc.scalar.activation(u_loc8[0:pw, t, 32:42], r1[0:pw, :],
                                         AF.Copy, scale=64.0)

            def stage_strip(si, agu_in):
                _, _, t0, nt = STRIPS[si]
                nc.sync.dma_start(agu_in[:, t0:t0 + nt, :],
                                  u_loc8[:, t0:t0 + nt, :])

            def launch_ag_u(agu_in, k):
                agu_out = dram.tile([NC, 128, LT, UW], FP8, tag="agout",
                                    name=f"agout{k}", addr_space="Shared")
                nc.gpsimd.collective_compute(
                    "AllGather", ALU.bypass, replica_groups=RG,
                    ins=[agu_in[:]], outs=[agu_out[:]],
                )
                for c in range(NC):
                    eng = nc.gpsimd if c % 2 == 0 else nc.scalar
                    eng.dma_start(out=u_stat[:, c, :, :], in_=agu_out[c])

            def compute_eta(k, h16s):
                """eT[k] = tanh(Txk @ Wp[k] + bp[k]) @ (gamma[:,k]/3)."""
                pshs, htas = [], []
                for si, (s0, sw, _, _) in enumerate(STRIPS):
                    psh = psX.tile([128, 512], F32, space="PSUM", tag="aux",
                                   name=f"psh{k}_{si}")
                    nc.tensor.matmul(psh[0:RANK, 0:sw],
                                     wps[:, k * RANK:(k + 1) * RANK],
                                     h16s[si][:, 0:sw], start=True, stop=True)
                    hta = small.tile([RANK, 512], FP16, tag="hta",
                                     name=f"hta{k}_{si}")
                    nc.scalar.activation(hta[:, 0:sw], psh[0:RANK, 0:sw], AF.Tanh,
                                         bias=bps[:, k:k + 1], scale=1.0)
                    pshs.append(psh); htas.append(hta)
                eRow = small.tile([1, NSH], F32, tag="eRow", name=f"eRow{k}", bufs=1)
                for si, (s0, sw, _, _) in enumerate(STRIPS):
                    pse2 = psX.tile([128, 512], F32, space="PSUM", tag="aux",
                                    name=f"pse2{k}_{si}")
                    nc.tensor.matmul(pse2[0:1, 0:sw], gams[:, k:k + 1],
                                     htas[si][:, 0:sw], start=True, stop=True)
                    nc.vector.tensor_copy(eRow[:, s0:s0 + sw], pse2[0:1, 0:sw])
                nc.sync.dma_start(eT[k:k + 1, :], eRow[:])

            # ---------------- prologue ----------------
            agu_in = dram.tile([128, LT, UW], FP8, tag="agin", name="agin0")
            for si in range(3):
                prep_strip(si, st[0], "p")
                stage_strip(si, agu_in)
            launch_ag_u(agu_in, 0)
            compute_eta(0, h16_of)

            # ---------------- hops ----------------
            cur_i, prev_i, free_i = 0, None, 1
            for k in range(1, K + 1):
                # strip-major DoubleRow matmul burst (waits on u_stat DMA)
                pss = []
                for si, (s0, sw, _, _) in enumerate(STRIPS):
                    ps = psH.tile([42, 512], F32, space="PSUM", tag=f"s{si}",
                                  name=f"hop{k}s{si}")
                    pss.append(ps)
                for si, (s0, sw, _, _) in enumerate(STRIPS):
                    for cg in range(NC):
                        for i in range(PAIRS):
                            jg = cg * PAIRS + i
                            nc.tensor.matmul(
                                pss[si][:, 0:sw],
                                u_stat[:, cg, 2 * i:2 * i + 2, 0:42],
                                A8[:, cg, 2 * i:2 * i + 2, s0:s0 + sw],
                                start=(jg == 0), stop=(jg == NC * PAIRS - 1),
                                perf_mode=DR,
                            )
                # per-strip: evac + Chebyshev combine + next-u prep
                scale_rows = dB[1] if k == 1 else dB[2]
                nxt = st[free_i]
                if k < K:
                    agu_in = dram.tile([128, LT, UW], FP8, tag="agin",
                                       name=f"agin{k}")
                h16s = {}
                for si, (s0, sw, t0, nt) in enumerate(STRIPS):
                    ps = pss[si]
                    m1 = small.tile([C, 512], F32, tag="ev1", name=f"m1_{k}_{si}", bufs=2)
                    nc.scalar.activation(m1[:, 0:sw], ps[32:42, 0:sw], AF.Copy,
                                         scale=1.0 / 64.0)
                    a2 = small.tile([C, 512], F32, tag="ev2", name=f"a2_{k}_{si}", bufs=2)
                    nc.vector.tensor_tensor(out=a2[:, 0:sw], in0=ps[0:10, 0:sw],
                                            in1=m1[:, 0:sw], op=ALU.add)
                    if k == 1:
                        nc.vector.tensor_tensor(out=nxt[:, s0:s0 + sw],
                                                in0=a2[:, 0:sw],
                                                in1=scale_rows[:, s0:s0 + sw],
                                                op=ALU.mult)
                    else:
                        t3 = small.tile([C, 512], F32, tag="ev1",
                                        name=f"t3_{k}_{si}", bufs=2)
                        nc.vector.tensor_tensor(out=t3[:, 0:sw], in0=a2[:, 0:sw],
                                                in1=scale_rows[:, s0:s0 + sw],
                                                op=ALU.mult)
                        nc.vector.tensor_tensor(out=nxt[:, s0:s0 + sw],
                                                in0=t3[:, 0:sw],
                                                in1=st[prev_i][:, s0:s0 + sw],
                                                op=ALU.subtract)
                    h16 = small.tile([C, 512], FP16, tag="h16", name=f"h16_{k}_{si}")
                    nc.scalar.activation(h16[:, 0:sw], nxt[:, s0:s0 + sw], AF.Copy)
                    nc.sync.dma_start(hist_dram[k, :, s0:s0 + sw], h16[:, 0:sw])
                    h16s[si] = h16
                    if k < K:
                        prep_strip(si, nxt, f"h{k}")
                        stage_strip(si, agu_in)
                if DEBUG:
                    nc.sync.dma_start(dump_tx[k], nxt[:])
                prev_i, cur_i = cur_i, free_i
                free_i = 3 - cur_i - prev_i
                if k < K:
                    launch_ag_u(agu_in, k)
                compute_eta(k, h16s)

        # ---------------- E allgather ----------------
        with tc.tile_pool(name="psT2", bufs=2, space="PSUM") as psT2, \
             tc.tile_pool(name="tailp", bufs=1) as tailp:
            etaS = tailp.tile([K + 1, NSH], F32, tag="etaS")
            hidT = tailp.tile([C, NSH], F32, tag="hidT")
            for t in range(LT):
                pw = 128 if t < LT - 1 else LLAST
                psE = psT2.tile([128, 512], F32, space="PSUM", tag="aux2",
                                name=f"psE{t}")
                nc.tensor.transpose(psE[0:pw, 0:K + 1],
                                    eT[:, t * 128:t * 128 + pw],
                                    ident[0:K + 1, 0:K + 1])
                nc.scalar.activation(e_loc[0:pw, t, 0:K + 1], psE[0:pw, 0:K + 1],
                                     AF.Copy)
            if DEBUG:
                nc.sync.dma_start(dump_e[:], eT[:])
            age_in = dram.tile([128, LT, EW], FP16, tag="agein", name="agein")
            age_out = dram.tile([NC, 128, LT, EW], FP16, tag="ageout",
                                name="ageout", addr_space="Shared")
            nc.sync.dma_start(age_in[:], e_loc[:])
            nc.gpsimd.collective_compute(
                "AllGather", ALU.bypass, replica_groups=RG,
                ins=[age_in[:]], outs=[age_out[:]],
            )
            for c in range(NC):
                eng = nc.gpsimd if c % 2 == 0 else nc.scalar
                eng.dma_start(out=e_stat[:, c, :, :], in_=age_out[c])

            # ---------------- CTC @ E ----------------
            with tc.tile_pool(name="psC", bufs=1, space="PSUM") as psC:
                pcs = [psC.tile([K + 1, 512], F32, space="PSUM", tag=f"c{si}",
                                name=f"ctc{si}") for si in range(3)]
                for jg in range(GJT):
                    cg, t = jg // LT, jg % LT
                    kw = 128 if t < LT - 1 else LLAST
                    row0 = cg * NSH + t * 128
                    cj = stream.tile([128, NSH], FP16, tag="mv", name=f"cj{jg}")
                    qeng = (nc.sync, nc.scalar)[jg % 2]
                    qeng.dma_start(cj[0:kw, :], ctct_dram[row0:row0 + kw, :])
                    for si, (s0, sw, _, _) in enumerate(STRIPS):
                        nc.tensor.matmul(
                            pcs[si][:, 0:sw], e_stat[0:kw, cg, t, 0:K + 1],
                            cj[0:kw, s0:s0 + sw],
                            start=(jg == 0), stop=(jg == GJT - 1),
                        )
                for si, (s0, sw, _, _) in enumerate(STRIPS):
                    nc.vector.tensor_copy(etaS[:, s0:s0 + sw], pcs[si][:, 0:sw])
            if DEBUG:
                nc.sync.dma_start(dump_eta[:], etaS[:])

            # ---------------- hidden = sum_k Txk * eta_k ----------------
            # hidden = sum_k Txk * eta_k, PE-heavy formulation:
            #   psb[110, s] = sel11^T @ etaS      (row k broadcast to 10 rows)
            #   prod = hist110 * psb  (one 110-partition DVE op, fp16 out)
            #   hid[c, s] = sel11T^T @ prod       (sum over k groups)
            for si, (s0, sw, _, _) in enumerate(STRIPS):
                h110 = tailp.tile([(K + 1) * C, 512], FP16, tag="h110",
                                  name=f"h110_{si}", bufs=2)
                nc.sync.dma_start(
                    h110[:, 0:sw],
                    hist_dram[:, :, s0:s0 + sw].rearrange("k c s -> (k c) s"))
                psb = psT2.tile([128, 512], F32, space="PSUM", tag="aux2",
                                name=f"psb{si}")
                nc.tensor.matmul(psb[0:(K + 1) * C, 0:sw],
                                 sel11s[:, 0:(K + 1) * C],
                                 etaS[:, s0:s0 + sw], start=True, stop=True)
                prod = tailp.tile([(K + 1) * C, 512], FP16, tag="prod",
                                  name=f"prod{si}", bufs=2)
                nc.vector.tensor_tensor(out=prod[:, 0:sw], in0=h110[:, 0:sw],
                                        in1=psb[0:(K + 1) * C, 0:sw],
                                        op=ALU.mult)
                ps10 = psT2.tile([128, 512], F32, space="PSUM", tag="aux2",
                                 name=f"ps10_{si}")
                nc.tensor.matmul(ps10[0:C, 0:sw], sel11Ts[:], prod[:, 0:sw],
                                 start=True, stop=True)
                nc.scalar.activation(hidT[:, s0:s0 + sw], ps10[0:C, 0:sw],
                                     AF.Copy)
            if DEBUG:
                nc.sync.dma_start(dump_hid[:], hidT[:])

            # ---------------- log_softmax + out ----------------
            for t in range(LT):
                pw = 128 if t < LT - 1 else LLAST
                psS = psT2.tile([128, 512], F32, space="PSUM", tag="aux2",
                                name=f"psS{t}")
                nc.tensor.transpose(psS[0:pw, 0:C], hidT[:, t * 128:t * 128 + pw],
                                    ident[0:C, 0:C])
                h = small.tile([128, C], F32, tag="hrow", name=f"hrow{t}")
                nc.vector.tensor_copy(h[0:pw, :], psS[0:pw, 0:C])
                mx = small.tile([128, 1], F32, tag="mx", name=f"mx{t}")
                nc.vector.tensor_reduce(mx[0:pw, :], h[0:pw, :],
                                        axis=mybir.AxisListType.X, op=ALU.max)
                sh = small.tile([128, C], F32, tag="sh", name=f"sh{t}")
                nc.vector.tensor_scalar_sub(sh[0:pw, :], h[0:pw, :], mx[0:pw, :])
                ex = small.tile([128, C], F32, tag="ex", name=f"ex{t}")
                sm = small.tile([128, 1], F32, tag="sm", name=f"sm{t}")
                nc.scalar.activation(ex[0:pw, :], sh[0:pw, :], AF.Exp,
                                     accum_out=sm[0:pw, :])
                ls = small.tile([128, 1], F32, tag="ls", name=f"ls{t}")
                nc.scalar.activation(ls[0:pw, :], sm[0:pw, :], AF.Ln)
                o = small.tile([128, C], F32, tag="o", name=f"o{t}")
                nc.vector.tensor_scalar_sub(o[0:pw, :], sh[0:pw, :], ls[0:pw, :])
                nc.sync.dma_start(out_dram[t * 128:t * 128 + pw, :], o[0:pw, :])

    nc.compile()
    return nc


def _host_prep(feature, edges, CTC, W1, b1, W2, b2, gamma, Wp, bp):
    src = np.asarray(edges[0], dtype=np.int64)
    dst = np.asarray(edges[1], dtype=np.int64)
    nonself = src != dst
    s, d = src[nonself], dst[nonself]

    deg = np.bincount(s, minlength=N).astype(np.float64)
    dinv = np.where(deg > 0, 1.0 / np.sqrt(np.maximum(deg, 1e-30)), 0.0).astype(np.float32)

    counts = np.zeros((N, N), dtype=np.uint8)
    np.add.at(counts, (s, d), 1)
    lut = np.arange(256).astype(NP_FP8)
    a8 = lut[counts]          # [N, N] fp8, exact small ints

    feature = np.asarray(feature, dtype=np.float32)
    CTC = np.asarray(CTC, dtype=np.float32)

    sel3 = np.zeros((3, 30), dtype=np.float32)
    for r in range(3):
        sel3[r, r * 10:(r + 1) * 10] = 1.0
    sel11 = np.zeros((K + 1, (K + 1) * C), dtype=np.float32)
    for r in range(K + 1):
        sel11[r, r * C:(r + 1) * C] = 1.0
    sel11t = np.zeros(((K + 1) * C, C), dtype=np.float16)
    for r in range(K + 1):
        for c in range(C):
            sel11t[r * C + c, c] = 1.0

    in_maps = []
    for k in range(NC):
        r0, r1 = k * NSH, (k + 1) * NSH
        dloc = dinv[r0:r1]
        dinvs = np.stack([dloc, -dloc, -2.0 * dloc]).astype(np.float32)
        in_maps.append({
            "a8": np.ascontiguousarray(a8[:, r0:r1]),
            "featT": np.ascontiguousarray(feature[r0:r1].T.astype(np.float16)),
            "ctct": np.ascontiguousarray(CTC[r0:r1].astype(np.float16).T),
            "w1": np.asarray(W1, dtype=np.float16),
            "b1": np.asarray(b1, dtype=np.float32).reshape(HID, 1),
            "w2": np.asarray(W2, dtype=np.float16),
            "b2": np.asarray(b2, dtype=np.float32).reshape(C, 1),
            "wp": np.ascontiguousarray(np.asarray(Wp, dtype=np.float32).transpose(1, 0, 2).reshape(C, (K + 1) * RANK)).astype(np.float16),
            "bp": np.ascontiguousarray(np.asarray(bp, dtype=np.float32).T),
            "gam": (np.asarray(gamma, dtype=np.float32) / RANK).astype(np.float16),
            "dinvs": dinvs,
            "sel3": sel3,
            "sel11": sel11,
            "sel11t": sel11t,
        })
    return in_maps


def kernel(feature, edges, CTC, W1, b1, W2, b2, gamma, Wp, bp):
    from concourse.bass_utils import run_bass_kernel_spmd

    if "nc" not in _CACHE:
        _CACHE["nc"] = _build_program()
    nc = _CACHE["nc"]

    in_maps = _host_prep(feature, edges, CTC, W1, b1, W2, b2, gamma, Wp, bp)
    trace = bool(os.environ.get("GNN_TRACE"))
    res = run_bass_kernel_spmd(nc, in_maps, list(range(NC)), trace=trace)
    _CACHE["last_result"] = res
    out = np.concatenate([res.results[k]["out"] for k in range(NC)], axis=0)
    return out.astype(np.float32)
